# revision 19
# baseline (speedup 1.0000x reference)
"""Distributed triangle multiplication (AlphaFold-style) for 8 Trainium2 NeuronCores.

v3 per-core pipeline (host row-shards pair as bf16, 96 rows/core):
  p1 pass1: batched bn_stats (4 j-subtiles per DVE instr) for all blocks;
    one Act Rsqrt + ~10 batched DVE ops combine the even/odd partial
    stats into LN1 scale/shift for all 73728 positions.
  p1 pass2 (per block): reload x, normalize (Act Identity / DVE
    tensor_scalar split), DMA-crossbar transpose into resident xT,
    and immediately run channel-group-0 projection+gate matmuls,
    batched sigmoid, fused product, store to ab_i[0].
  CC#1(0) fires at p1 end; group-1 projection overlaps it.
  xT spilled to DRAM; SBUF freed for deep phase-2 staging (bufs=4).
  p2 group 0 staged via crossbar right after CC#1(0) (created before
    CC#1(1) so Tile's collective->transpose serialization does not
    gate it); CC#1(1) overlaps group-0 einsum; CC#2a (channel half)
    overlaps group-1 einsum; CC#2b overlaps phase-3 stats part 1.
  p3: split-channel bn_stats with manual even/odd recombination, one
    Rsqrt; passB renormalizes, crossbars back to channel-major, out &
    gating matmuls vs reloaded xT, batched sigmoid, fused final mul,
    f32 channel-major store (host transposes).
Activation table sets never interleave (rsqrt-set vs sigmoid-set) ->
4 table loads total.
"""
import sys

for _p in ("/opt/trn_rl_repo", "/opt/trn_rl_repo/concourse"):
    if _p not in sys.path:
        sys.path.insert(0, _p)

import os
import numpy as np
import ml_dtypes

import concourse.bass as bass
import concourse.tile as tile
from concourse import bacc, mybir
from concourse.bass_utils import run_bass_kernel_spmd

F32 = mybir.dt.float32
BF16 = mybir.dt.bfloat16
AF = mybir.ActivationFunctionType
ALU = mybir.AluOpType

N = 768
C = 128
NCORES = 8
R = N // NCORES            # 96 rows per core
POS = R * N                # 73728 positions per core
EPS = 1e-5
GS = 16                    # 16 x 128-pos subtiles per block
NG = POS // (GS * 128)     # 36 blocks of 2048 positions
P1_DVE_J = 8               # p1 normalize j-subtiles on DVE (rest Act)
P3_DVE_J = 10              # p3 normalize split

_PROGRAM_CACHE = {}
LAST_EXEC_NS = None
LAST_TRACE = None


def _build_program(use_mask, sim=False, dbg=False):
    nc = bacc.Bacc("TRN2", target_bir_lowering=False, debug=False,
                   num_devices=1 if sim else NCORES)

    def _collective(ins, outs):
        if sim:
            nc.sync.dma_start(out=outs[0], in_=ins[0])
        else:
            nc.gpsimd.collective_compute("AllToAll", ALU.bypass,
                                         [list(range(NCORES))],
                                         ins=ins, outs=outs)

    pair_r = nc.dram_tensor("pair_r", [POS, C], BF16, kind="ExternalInput").ap()
    w1t = nc.dram_tensor("w1t", [C, 512], BF16, kind="ExternalInput").ap()
    wfin = nc.dram_tensor("wfin", [C, 2 * C], BF16, kind="ExternalInput").ap()
    c1r = nc.dram_tensor("c1r", [1, 512], F32, kind="ExternalInput").ap()
    cfr = nc.dram_tensor("cfr", [1, 256], F32, kind="ExternalInput").ap()
    if use_mask:
        mask_r = nc.dram_tensor("mask_r", [POS // 1024, 1024], F32,
                                kind="ExternalInput").ap()

    ab_i = [nc.dram_tensor(f"ab{q}_i", [128, POS], BF16).ap() for q in range(2)]
    ab_o = [nc.dram_tensor(f"ab{q}_o", [128, POS], BF16).ap() for q in range(2)]
    o2_i = [nc.dram_tensor(f"o2{q}_i", [64, POS], BF16).ap() for q in range(2)]
    o2_o = [nc.dram_tensor(f"o2{q}_o", [64, POS], BF16).ap() for q in range(2)]
    xT_d = nc.dram_tensor("xT_d", [C, POS], BF16).ap()
    out_r = nc.dram_tensor("out_r", [C, POS], F32, kind="ExternalOutput").ap()
    if dbg:
        dbg_xT = nc.dram_tensor("dbg_xT", [C, POS], BF16,
                                kind="ExternalOutput").ap()
        dbg_ab0i = nc.dram_tensor("dbg_ab0i", [128, POS], BF16,
                                  kind="ExternalOutput").ap()
        dbg_o2i = nc.dram_tensor("dbg_o2i", [64, POS], BF16,
                                 kind="ExternalOutput").ap()

    def stats_combine(st_means, st_cvs, rr_out, nm_out, pool, eng, epsb,
                      shape):
        """Combine k equal-size bn_stats groups (lists of [128,X,1]-shaped
        APs) into rr = rsqrt(var+eps), nm = -mean*rr (both [128, X])."""
        k = len(st_means)
        sm = pool.tile(shape, F32, tag="sc_sm")
        nc.vector.tensor_add(sm[:], st_means[0], st_means[1])
        for i in range(2, k):
            nc.vector.tensor_add(sm[:], sm[:], st_means[i])
        s2 = pool.tile(shape, F32, tag="sc_s2")
        t = pool.tile(shape, F32, tag="sc_t")
        nc.vector.tensor_mul(s2[:], st_means[0], st_means[0])
        for i in range(1, k):
            nc.vector.tensor_mul(t[:], st_means[i], st_means[i])
            nc.vector.tensor_add(s2[:], s2[:], t[:])
        scv = pool.tile(shape, F32, tag="sc_scv")
        nc.vector.tensor_add(scv[:], st_cvs[0], st_cvs[1])
        for i in range(2, k):
            nc.vector.tensor_add(scv[:], scv[:], st_cvs[i])
        # var = scv/128 + s2/k - mean^2 ; mean = sm/k
        nc.vector.tensor_scalar_mul(s2[:], s2[:], 1.0 / k)
        var = pool.tile(shape, F32, tag="sc_var")
        nc.vector.scalar_tensor_tensor(var[:], scv[:], 1.0 / 128.0, s2[:],
                                       ALU.mult, ALU.add)
        mbar = pool.tile(shape, F32, tag="sc_mb")
        nc.vector.tensor_scalar_mul(mbar[:], sm[:], 1.0 / k)
        nc.vector.tensor_mul(t[:], mbar[:], mbar[:])
        nc.vector.tensor_sub(var[:], var[:], t[:])
        nc.scalar.activation(t[:], var[:], AF.Sqrt, bias=epsb[:])
        nc.vector.reciprocal(rr_out, t[:])
        nc.vector.scalar_tensor_tensor(nm_out, mbar[:], -1.0, rr_out,
                                       ALU.mult, ALU.mult)

    with tile.TileContext(nc) as tc:
        with tc.tile_pool(name="consts", bufs=1) as cpool:
            w1sb = cpool.tile([C, 512], BF16)
            nc.sync.dma_start(w1sb[:], w1t[:, :])
            wfsb = cpool.tile([C, 2 * C], BF16)
            nc.sync.dma_start(wfsb[:], wfin[:, :])
            c1sb = cpool.tile([128, 4], F32)
            nc.sync.dma_start(
                c1sb[:], c1r[0, :].rearrange("(a p) -> p a", p=128))
            cfsb = cpool.tile([128, 2], F32)
            nc.sync.dma_start(
                cfsb[:], cfr[0, :].rearrange("(a p) -> p a", p=128))
            epsb = cpool.tile([128, 1], F32)
            nc.vector.memset(epsb[:], EPS)

            # ================= Phase 1 =================
            def p1b_unit(q, g, h2, p1ps, p1sig, p1pr, xTap):
                pos0 = g * 2048 + h2 * 1024
                ps = p1ps.tile([128, 2, 2, 512], F32, tag="ps")
                for kind in range(2):
                    w0 = q * 256 + kind * 128
                    for hb in range(2):
                        nc.tensor.matmul(
                            ps[:, kind, hb, :], w1sb[:, w0:w0 + 128],
                            xTap[:, pos0 + hb * 512:pos0 + (hb + 1) * 512],
                            start=True, stop=True)
                sig = p1sig.tile([128, 2, 512], BF16, tag="sig")
                nc.scalar.activation(sig[:], ps[:, 1, :, :], AF.Sigmoid,
                                     bias=c1sb[:, 2 * q + 1:2 * q + 2])
                prod = p1pr.tile([128, 2, 512], BF16, tag="prod")
                nc.vector.scalar_tensor_tensor(
                    prod[:], ps[:, 0, :, :], c1sb[:, 2 * q:2 * q + 1],
                    sig[:], ALU.add, ALU.mult)
                if use_mask:
                    mrow = p1sig.tile([1, 1024], F32, tag="mrow")
                    nc.sync.dma_start(
                        mrow[:], mask_r[pos0 // 1024:pos0 // 1024 + 1, :])
                    mb = p1sig.tile([128, 1024], F32, tag="mb")
                    nc.gpsimd.partition_broadcast(mb[:], mrow[:])
                    nc.vector.tensor_mul(
                        prod[:], prod[:],
                        mb[:].rearrange("p (a b) -> p a b", a=2))
                nc.sync.dma_start(ab_i[q][:, pos0:pos0 + 1024],
                                  prod[:].rearrange("p a b -> p (a b)"))

            with tc.tile_pool(name="xTp", bufs=1) as xTp, \
                 tc.tile_pool(name="p1x", bufs=2) as p1x, \
                 tc.tile_pool(name="p1st", bufs=1) as p1st, \
                 tc.tile_pool(name="p1xn", bufs=2) as p1xn, \
                 tc.tile_pool(name="p1ps", bufs=2, space="PSUM") as p1ps, \
                 tc.tile_pool(name="p1sig", bufs=3) as p1sig, \
                 tc.tile_pool(name="p1pr", bufs=3) as p1pr:
                xT = xTp.tile([C, POS], BF16)
                # ---- pass 1: stats ----
                st1 = p1st.tile([128, NG, GS, 6], F32)
                for g in range(NG):
                    xt8 = p1x.tile([128, GS, C], BF16, tag="xt8")
                    nc.sync.dma_start(
                        xt8[:],
                        pair_r[g * GS * 128:(g + 1) * GS * 128, :].rearrange(
                            "(s p) c -> p s c", p=128))
                    for j in range(GS):
                        nc.vector.bn_stats(st1[:, g, j, :], xt8[:, j, :])
                sh = [128, NG, GS, 1]
                stv = st1[:]
                rr1 = p1st.tile([128, NG, GS], F32)
                nm1 = p1st.tile([128, NG, GS], F32)
                stats_combine(
                    [stv[:, :, :, 1:2], stv[:, :, :, 4:5]],
                    [stv[:, :, :, 2:3], stv[:, :, :, 5:6]],
                    rr1[:].rearrange("p g (s o) -> p g s o", o=1),
                    nm1[:].rearrange("p g (s o) -> p g s o", o=1),
                    p1st, nc, epsb, sh)
                rr1f = rr1[:].rearrange("p g s -> p (g s)")
                nm1f = nm1[:].rearrange("p g s -> p (g s)")
                # ---- pass 2: normalize + transpose + group-0 proj ----
                for g in range(NG):
                    xt8 = p1x.tile([128, GS, C], BF16, tag="xt8")
                    nc.sync.dma_start(
                        xt8[:],
                        pair_r[g * GS * 128:(g + 1) * GS * 128, :].rearrange(
                            "(s p) c -> p s c", p=128))
                    xn8 = p1xn.tile([128, GS, C], BF16, tag="xn8")
                    for j in range(GS):
                        jj = g * GS + j
                        if j < P1_DVE_J:
                            nc.vector.tensor_scalar(
                                xn8[:, j, :], xt8[:, j, :],
                                rr1f[:, jj:jj + 1], nm1f[:, jj:jj + 1],
                                ALU.mult, ALU.add)
                        else:
                            nc.scalar.activation(
                                xn8[:, j, :], xt8[:, j, :], AF.Identity,
                                bias=nm1f[:, jj:jj + 1],
                                scale=rr1f[:, jj:jj + 1])
                    nc.sync.dma_start_transpose(
                        xT[:, g * GS * 128:(g + 1) * GS * 128].rearrange(
                            "c (s p) -> c s p", s=GS), xn8[:])
                    for h2 in range(2):
                        p1b_unit(0, g, h2, p1ps, p1sig, p1pr, xT)
                _collective([ab_i[0][:]], [ab_o[0][:]])
                for g in range(NG):
                    for h2 in range(2):
                        p1b_unit(1, g, h2, p1ps, p1sig, p1pr, xT)
                nc.sync.dma_start(xT_d[:, :], xT[:])
                if dbg:
                    nc.sync.dma_start(dbg_xT[:, :], xT[:])

            # ================= Phase 2 =================
            KC = N // 128

            def p2_channel(q, s, p2a, p2b, p2o, p2mm):
                AT = p2a.tile([128, KC, N], BF16, tag="AT")
                BT = p2b.tile([128, KC, N], BF16, tag="BT")
                for d in range(NCORES):
                    for which, Tt in ((0, AT), (1, BT)):
                        nc.sync.dma_start_transpose(
                            Tt[:, :, d * R:(d + 1) * R],
                            ab_o[q][16 * d + 2 * s + which, :].rearrange(
                                "(i k) -> i k", i=R))
                for ib in range(6):
                    ps2 = p2mm.tile([128, 2, 512], F32, tag="ps2")
                    for kc in range(KC):
                        lhsT = AT[:, kc, ib * 128:(ib + 1) * 128]
                        for jh in range(2):
                            nc.tensor.matmul(
                                ps2[:, jh, 0:384], lhsT,
                                BT[:, kc, jh * 384:(jh + 1) * 384],
                                start=(kc == 0), stop=(kc == KC - 1))
                    ot = p2o.tile([128, 2, 384], BF16, tag="ot")
                    if ib % 2 == 0:
                        nc.scalar.activation(ot[:], ps2[:, :, 0:384], AF.Copy)
                    else:
                        nc.vector.tensor_copy(ot[:], ps2[:, :, 0:384])
                    otv = ot[:].rearrange("i h j -> i (h j)")
                    i0 = ib * 128
                    while i0 < (ib + 1) * 128:
                        d, off = divmod(i0, R)
                        n = min(R - off, (ib + 1) * 128 - i0)
                        nc.scalar.dma_start(
                            o2_i[q][8 * d + s, :].rearrange(
                                "(i j) -> i j", i=R)[off:off + n, :],
                            otv[i0 - ib * 128:i0 - ib * 128 + n, :])
                        i0 += n

            with tc.tile_pool(name="p2a", bufs=4) as p2a, \
                 tc.tile_pool(name="p2b", bufs=4) as p2b, \
                 tc.tile_pool(name="p2o", bufs=4) as p2o, \
                 tc.tile_pool(name="p2mm", bufs=2, space="PSUM") as p2mm:
                for s in range(8):
                    p2_channel(0, s, p2a, p2b, p2o, p2mm)
                _collective([ab_i[1][:]], [ab_o[1][:]])
                for s in range(8):
                    p2_channel(1, s, p2a, p2b, p2o, p2mm)
                _collective([o2_i[0][:]], [o2_o[0][:]])

            # ================= Phase 3 =================
            with tc.tile_pool(name="p3rr", bufs=1) as p3rr, \
                 tc.tile_pool(name="p3ps", bufs=2, space="PSUM") as p3ps:
                rr3 = p3rr.tile([128, NG, GS], F32)
                nm3 = p3rr.tile([128, NG, GS], F32)
                with tc.tile_pool(name="p3a", bufs=3) as p3a, \
                     tc.tile_pool(name="p3st", bufs=1) as p3st:
                    st3 = p3st.tile([128, 2, NG, GS, 6], F32)
                    for g in range(NG):
                        o2p = p3a.tile([128, GS, 64], BF16, tag="o2pa")
                        nc.sync.dma_start_transpose(
                            o2p[:],
                            o2_o[0][:, g * GS * 128:(g + 1) * GS * 128])
                        for j in range(GS):
                            nc.vector.bn_stats(st3[:, 0, g, j, :],
                                               o2p[:, j, :])
                    _collective([o2_i[1][:]], [o2_o[1][:]])
                    for g in range(NG):
                        o2p = p3a.tile([128, GS, 64], BF16, tag="o2pb")
                        nc.sync.dma_start_transpose(
                            o2p[:],
                            o2_o[1][:, g * GS * 128:(g + 1) * GS * 128])
                        for j in range(GS):
                            nc.vector.bn_stats(st3[:, 1, g, j, :],
                                               o2p[:, j, :])
                    sh = [128, NG, GS, 1]
                    sta = st3[:, 0]
                    stb = st3[:, 1]
                    stats_combine(
                        [sta[:, :, :, 1:2], sta[:, :, :, 4:5],
                         stb[:, :, :, 1:2], stb[:, :, :, 4:5]],
                        [sta[:, :, :, 2:3], sta[:, :, :, 5:6],
                         stb[:, :, :, 2:3], stb[:, :, :, 5:6]],
                        rr3[:].rearrange("p g (s o) -> p g s o", o=1),
                        nm3[:].rearrange("p g (s o) -> p g s o", o=1),
                        p3st, nc, epsb, sh)
                rr3f = rr3[:].rearrange("p g s -> p (g s)")
                nm3f = nm3[:].rearrange("p g s -> p (g s)")
                _cms = [tc.tile_pool(name="xTp3", bufs=1),
                        tc.tile_pool(name="p3b", bufs=2),
                        tc.tile_pool(name="p3n", bufs=2),
                        tc.tile_pool(name="p3sig", bufs=2),
                        tc.tile_pool(name="p3om", bufs=2)]
                xTp3, p3b, p3n, p3sig, p3om = (cm.__enter__()
                                               for cm in _cms)
                xT3 = xTp3.tile([C, POS], BF16)
                nc.sync.dma_start(xT3[:], xT_d[:, :])
                for g in range(NG):
                    o2p = p3b.tile([128, GS, 128], BF16, tag="o2p")
                    for q in range(2):
                        nc.sync.dma_start_transpose(
                            o2p[:, :, 64 * q:64 * q + 64],
                            o2_o[q][:, g * GS * 128:(g + 1) * GS * 128])
                    o2n = p3n.tile([128, GS, 128], BF16, tag="o2n")
                    for j in range(GS):
                        jj = g * GS + j
                        if j < P3_DVE_J:
                            nc.vector.tensor_scalar(
                                o2n[:, j, :], o2p[:, j, :],
                                rr3f[:, jj:jj + 1], nm3f[:, jj:jj + 1],
                                ALU.mult, ALU.add)
                        else:
                            nc.scalar.activation(
                                o2n[:, j, :], o2p[:, j, :], AF.Identity,
                                bias=nm3f[:, jj:jj + 1],
                                scale=rr3f[:, jj:jj + 1])
                    o2nT = p3n.tile([128, GS, 128], BF16, tag="o2nT")
                    nc.sync.dma_start_transpose(o2nT[:], o2n[:])
                    o2v = o2nT[:].rearrange("c s p -> c (s p)")
                    for h2 in range(2):
                        pos0 = g * 2048 + h2 * 1024
                        ps3 = p3ps.tile([128, 2, 2, 512], F32, tag="ps3")
                        for hb in range(2):
                            sl = slice(h2 * 1024 + hb * 512,
                                       h2 * 1024 + (hb + 1) * 512)
                            nc.tensor.matmul(ps3[:, 0, hb, :],
                                             wfsb[:, 0:C], o2v[:, sl],
                                             start=True, stop=True)
                            nc.tensor.matmul(ps3[:, 1, hb, :],
                                             wfsb[:, C:2 * C],
                                             xT3[:, pos0 + hb * 512:
                                                 pos0 + (hb + 1) * 512],
                                             start=True, stop=True)
                        sigb = p3sig.tile([128, 2, 512], BF16, tag="sigb")
                        nc.scalar.activation(sigb[:], ps3[:, 1, :, :],
                                             AF.Sigmoid, bias=cfsb[:, 1:2])
                        om = p3om.tile([128, 1024], F32, tag="om")
                        nc.vector.scalar_tensor_tensor(
                            om[:].rearrange("p (a b) -> p a b", a=2),
                            ps3[:, 0, :, :], cfsb[:, 0:1], sigb[:],
                            ALU.add, ALU.mult)
                        nc.scalar.dma_start(out_r[:, pos0:pos0 + 1024], om[:])
                for _cm in reversed(_cms):
                    _cm.__exit__(None, None, None)
            if dbg:
                nc.sync.dma_start(dbg_ab0i[:, :], ab_i[0][:, :])
                nc.sync.dma_start(dbg_o2i[:, :], o2_i[0][:, :])
    nc.compile()
    return nc


def _perm_pi():
    """p3 o2nT partition p -> einsum channel c."""
    p = np.arange(128)
    q = p // 64
    d = (p % 64) // 8
    s = p % 8
    return d * 16 + q * 8 + s


def _prep_weights(ln1_w, proj_w, gate_w, ln2_w, out_w, gating_w, ln1_b, ln2_b):
    p = np.arange(128)
    d, u = p // 16, p % 16
    w1 = np.zeros((C, 512), np.float32)
    c1 = np.zeros(512, np.float32)
    for q in range(2):
        c = d * 16 + q * 8 + u // 2
        row = 2 * c + (u % 2)
        w1[:, q * 256 + 0:q * 256 + 128] = (proj_w[row] * ln1_w).T
        w1[:, q * 256 + 128:q * 256 + 256] = (gate_w[row] * ln1_w).T
        c1[q * 256:q * 256 + 128] = proj_w[row] @ ln1_b
        c1[q * 256 + 128:q * 256 + 256] = gate_w[row] @ ln1_b
    pi = _perm_pi()
    wfo = (out_w * ln2_w[None, :]).T[pi]       # rows permuted to o2nT order
    wfg = (gating_w * ln1_w[None, :]).T
    wf = np.concatenate([wfo, wfg], axis=1)
    cf = np.concatenate([out_w @ ln2_b, gating_w @ ln1_b])
    return w1, c1.reshape(1, 512), wf, cf.reshape(1, 256)


def kernel(pair, mask, ln1_w, ln1_b, proj_w, gate_w, ln2_w, ln2_b, out_w,
           gating_w):
    pair = np.asarray(pair, dtype=np.float32)
    mask = np.asarray(mask, dtype=np.float32)
    ln1_w = np.asarray(ln1_w, np.float32); ln1_b = np.asarray(ln1_b, np.float32)
    ln2_w = np.asarray(ln2_w, np.float32); ln2_b = np.asarray(ln2_b, np.float32)
    proj_w = np.asarray(proj_w, np.float32)
    gate_w = np.asarray(gate_w, np.float32)
    out_w = np.asarray(out_w, np.float32)
    gating_w = np.asarray(gating_w, np.float32)

    use_mask = not bool(np.all(mask == 1.0))
    if use_mask not in _PROGRAM_CACHE:
        _PROGRAM_CACHE[use_mask] = _build_program(use_mask)
    nc = _PROGRAM_CACHE[use_mask]

    w1, c1t, wf, cft = _prep_weights(ln1_w, proj_w, gate_w, ln2_w, out_w,
                                     gating_w, ln1_b, ln2_b)
    bf = ml_dtypes.bfloat16
    pair_b = pair.astype(bf).reshape(NCORES, POS, C)
    w1_b = np.ascontiguousarray(w1).astype(bf)
    wf_b = np.ascontiguousarray(wf).astype(bf)

    in_maps = []
    for c in range(NCORES):
        m = {
            "pair_r": pair_b[c],
            "w1t": w1_b,
            "wfin": wf_b,
            "c1r": c1t,
            "cfr": cft,
        }
        if use_mask:
            m["mask_r"] = np.ascontiguousarray(
                mask[c * R:(c + 1) * R].reshape(POS // 1024, 1024))
        in_maps.append(m)

    trace = os.environ.get("TRIMUL_TRACE", "") == "1"
    res = run_bass_kernel_spmd(nc, in_maps, core_ids=list(range(NCORES)),
                               trace=trace)
    global LAST_EXEC_NS, LAST_TRACE
    if res.exec_time_ns is not None:
        LAST_EXEC_NS = res.exec_time_ns
    if res.instructions_and_trace is not None:
        LAST_TRACE = res.instructions_and_trace[1]
    out = np.empty((N, N, C), np.float32)
    for c in range(NCORES):
        out[c * R:(c + 1) * R] = res.results[c]["out_r"].T.reshape(R, N, C)
    return out


# revision 20
# speedup vs baseline: 1.1976x; 1.1976x over previous
"""Distributed triangle multiplication (AlphaFold-style) for 8 Trainium2 NeuronCores.

v3 per-core pipeline (host row-shards pair as bf16, 96 rows/core):
  p1 pass1: batched bn_stats (4 j-subtiles per DVE instr) for all blocks;
    one Act Rsqrt + ~10 batched DVE ops combine the even/odd partial
    stats into LN1 scale/shift for all 73728 positions.
  p1 pass2 (per block): reload x, normalize (Act Identity / DVE
    tensor_scalar split), DMA-crossbar transpose into resident xT,
    and immediately run channel-group-0 projection+gate matmuls,
    batched sigmoid, fused product, store to ab_i[0].
  CC#1(0) fires at p1 end; group-1 projection overlaps it.
  xT spilled to DRAM; SBUF freed for deep phase-2 staging (bufs=4).
  p2 group 0 staged via crossbar right after CC#1(0) (created before
    CC#1(1) so Tile's collective->transpose serialization does not
    gate it); CC#1(1) overlaps group-0 einsum; CC#2a (channel half)
    overlaps group-1 einsum; CC#2b overlaps phase-3 stats part 1.
  p3: split-channel bn_stats with manual even/odd recombination, one
    Rsqrt; passB renormalizes, crossbars back to channel-major, out &
    gating matmuls vs reloaded xT, batched sigmoid, fused final mul,
    f32 channel-major store (host transposes).
Activation table sets never interleave (rsqrt-set vs sigmoid-set) ->
4 table loads total.
"""
import sys

for _p in ("/opt/trn_rl_repo", "/opt/trn_rl_repo/concourse"):
    if _p not in sys.path:
        sys.path.insert(0, _p)

import os
import numpy as np
import ml_dtypes

import concourse.bass as bass
import concourse.tile as tile
from concourse import bacc, mybir
from concourse.bass_utils import run_bass_kernel_spmd

F32 = mybir.dt.float32
BF16 = mybir.dt.bfloat16
AF = mybir.ActivationFunctionType
ALU = mybir.AluOpType

N = 768
C = 128
NCORES = 8
R = N // NCORES            # 96 rows per core
POS = R * N                # 73728 positions per core
EPS = 1e-5
GS = 16                    # 16 x 128-pos subtiles per block
NG = POS // (GS * 128)     # 36 blocks of 2048 positions
P1_DVE_J = 8               # p1 normalize j-subtiles on DVE (rest Act)
P3_DVE_J = 10              # p3 normalize split

_PROGRAM_CACHE = {}
LAST_EXEC_NS = None
LAST_TRACE = None


def _build_program(use_mask, sim=False, dbg=False):
    nc = bacc.Bacc("TRN2", target_bir_lowering=False, debug=False,
                   num_devices=1 if sim else NCORES)

    def _collective(ins, outs):
        if sim:
            nc.sync.dma_start(out=outs[0], in_=ins[0])
        else:
            nc.gpsimd.collective_compute("AllToAll", ALU.bypass,
                                         [list(range(NCORES))],
                                         ins=ins, outs=outs)

    pair_r = nc.dram_tensor("pair_r", [POS, C], BF16, kind="ExternalInput").ap()
    w1t = nc.dram_tensor("w1t", [C, 512], BF16, kind="ExternalInput").ap()
    wfin = nc.dram_tensor("wfin", [C, 2 * C], BF16, kind="ExternalInput").ap()
    c1r = nc.dram_tensor("c1r", [1, 512], F32, kind="ExternalInput").ap()
    cfr = nc.dram_tensor("cfr", [1, 256], F32, kind="ExternalInput").ap()
    if use_mask:
        mask_r = nc.dram_tensor("mask_r", [POS // 1024, 1024], F32,
                                kind="ExternalInput").ap()

    ab_i = [nc.dram_tensor(f"ab{q}_i", [128, POS], BF16).ap() for q in range(2)]
    ab_o = [nc.dram_tensor(f"ab{q}_o", [128, POS], BF16).ap() for q in range(2)]
    o2_i = [nc.dram_tensor(f"o2{q}_i", [64, POS], BF16).ap() for q in range(2)]
    o2_o = [nc.dram_tensor(f"o2{q}_o", [64, POS], BF16).ap() for q in range(2)]
    xT_d = nc.dram_tensor("xT_d", [C, POS], BF16).ap()
    out_r = nc.dram_tensor("out_r", [C, POS], F32, kind="ExternalOutput").ap()
    if dbg:
        dbg_xT = nc.dram_tensor("dbg_xT", [C, POS], BF16,
                                kind="ExternalOutput").ap()
        dbg_ab0i = nc.dram_tensor("dbg_ab0i", [128, POS], BF16,
                                  kind="ExternalOutput").ap()
        dbg_o2i = nc.dram_tensor("dbg_o2i", [64, POS], BF16,
                                 kind="ExternalOutput").ap()

    def stats_combine(st_means, st_cvs, rr_out, nm_out, pool, eng, epsb,
                      shape):
        """Combine k equal-size bn_stats groups (lists of [128,X,1]-shaped
        APs) into rr = rsqrt(var+eps), nm = -mean*rr (both [128, X])."""
        k = len(st_means)
        sm = pool.tile(shape, F32, tag="sc_sm")
        nc.vector.tensor_add(sm[:], st_means[0], st_means[1])
        for i in range(2, k):
            nc.vector.tensor_add(sm[:], sm[:], st_means[i])
        s2 = pool.tile(shape, F32, tag="sc_s2")
        t = pool.tile(shape, F32, tag="sc_t")
        nc.vector.tensor_mul(s2[:], st_means[0], st_means[0])
        for i in range(1, k):
            nc.vector.tensor_mul(t[:], st_means[i], st_means[i])
            nc.vector.tensor_add(s2[:], s2[:], t[:])
        scv = pool.tile(shape, F32, tag="sc_scv")
        nc.vector.tensor_add(scv[:], st_cvs[0], st_cvs[1])
        for i in range(2, k):
            nc.vector.tensor_add(scv[:], scv[:], st_cvs[i])
        # var = scv/128 + s2/k - mean^2 ; mean = sm/k
        nc.vector.tensor_scalar_mul(s2[:], s2[:], 1.0 / k)
        var = pool.tile(shape, F32, tag="sc_var")
        nc.vector.scalar_tensor_tensor(var[:], scv[:], 1.0 / 128.0, s2[:],
                                       ALU.mult, ALU.add)
        mbar = pool.tile(shape, F32, tag="sc_mb")
        nc.vector.tensor_scalar_mul(mbar[:], sm[:], 1.0 / k)
        nc.vector.tensor_mul(t[:], mbar[:], mbar[:])
        nc.vector.tensor_sub(var[:], var[:], t[:])
        nc.scalar.activation(t[:], var[:], AF.Sqrt, bias=epsb[:])
        nc.vector.reciprocal(rr_out, t[:])
        nc.vector.scalar_tensor_tensor(nm_out, mbar[:], -1.0, rr_out,
                                       ALU.mult, ALU.mult)

    with tile.TileContext(nc) as tc:
        with tc.tile_pool(name="consts", bufs=1) as cpool:
            w1sb = cpool.tile([C, 512], BF16)
            nc.sync.dma_start(w1sb[:], w1t[:, :])
            wfsb = cpool.tile([C, 2 * C], BF16)
            nc.sync.dma_start(wfsb[:], wfin[:, :])
            c1sb = cpool.tile([128, 4], F32)
            nc.sync.dma_start(
                c1sb[:], c1r[0, :].rearrange("(a p) -> p a", p=128))
            cfsb = cpool.tile([128, 2], F32)
            nc.sync.dma_start(
                cfsb[:], cfr[0, :].rearrange("(a p) -> p a", p=128))
            epsb = cpool.tile([128, 1], F32)
            nc.vector.memset(epsb[:], EPS)

            # ================= Phase 1 =================
            def p1b_unit(q, g, h2, p1ps, p1sig, p1pr, xTap):
                pos0 = g * 2048 + h2 * 1024
                ps = p1ps.tile([128, 2, 2, 512], F32, tag="ps")
                for kind in range(2):
                    w0 = q * 256 + kind * 128
                    for hb in range(2):
                        nc.tensor.matmul(
                            ps[:, kind, hb, :], w1sb[:, w0:w0 + 128],
                            xTap[:, pos0 + hb * 512:pos0 + (hb + 1) * 512],
                            start=True, stop=True)
                sig = p1sig.tile([128, 2, 512], BF16, tag="sig")
                nc.scalar.activation(sig[:], ps[:, 1, :, :], AF.Sigmoid,
                                     bias=c1sb[:, 2 * q + 1:2 * q + 2])
                prod = p1pr.tile([128, 2, 512], BF16, tag="prod")
                nc.vector.scalar_tensor_tensor(
                    prod[:], ps[:, 0, :, :], c1sb[:, 2 * q:2 * q + 1],
                    sig[:], ALU.add, ALU.mult)
                if use_mask:
                    mrow = p1sig.tile([1, 1024], F32, tag="mrow")
                    nc.sync.dma_start(
                        mrow[:], mask_r[pos0 // 1024:pos0 // 1024 + 1, :])
                    mb = p1sig.tile([128, 1024], F32, tag="mb")
                    nc.gpsimd.partition_broadcast(mb[:], mrow[:])
                    nc.vector.tensor_mul(
                        prod[:], prod[:],
                        mb[:].rearrange("p (a b) -> p a b", a=2))
                nc.sync.dma_start(ab_i[q][:, pos0:pos0 + 1024],
                                  prod[:].rearrange("p a b -> p (a b)"))

            with tc.tile_pool(name="xTp", bufs=1) as xTp, \
                 tc.tile_pool(name="p1x", bufs=2) as p1x, \
                 tc.tile_pool(name="p1st", bufs=1) as p1st, \
                 tc.tile_pool(name="p1xn", bufs=2) as p1xn, \
                 tc.tile_pool(name="p1ps", bufs=2, space="PSUM") as p1ps, \
                 tc.tile_pool(name="p1sig", bufs=3) as p1sig, \
                 tc.tile_pool(name="p1pr", bufs=3) as p1pr:
                xT = xTp.tile([C, POS], BF16)
                # ---- pass 1: stats ----
                st1 = p1st.tile([128, NG, GS, 6], F32)
                for g in range(NG):
                    xt8 = p1x.tile([128, GS, C], BF16, tag="xt8")
                    nc.sync.dma_start(
                        xt8[:],
                        pair_r[g * GS * 128:(g + 1) * GS * 128, :].rearrange(
                            "(s p) c -> p s c", p=128))
                    for j in range(GS):
                        nc.vector.bn_stats(st1[:, g, j, :], xt8[:, j, :])
                sh = [128, NG, GS, 1]
                stv = st1[:]
                rr1 = p1st.tile([128, NG, GS], F32)
                nm1 = p1st.tile([128, NG, GS], F32)
                stats_combine(
                    [stv[:, :, :, 1:2], stv[:, :, :, 4:5]],
                    [stv[:, :, :, 2:3], stv[:, :, :, 5:6]],
                    rr1[:].rearrange("p g (s o) -> p g s o", o=1),
                    nm1[:].rearrange("p g (s o) -> p g s o", o=1),
                    p1st, nc, epsb, sh)
                rr1f = rr1[:].rearrange("p g s -> p (g s)")
                nm1f = nm1[:].rearrange("p g s -> p (g s)")
                # ---- pass 2: normalize + transpose + group-0 proj ----
                for g in range(NG):
                    xt8 = p1x.tile([128, GS, C], BF16, tag="xt8")
                    nc.sync.dma_start(
                        xt8[:],
                        pair_r[g * GS * 128:(g + 1) * GS * 128, :].rearrange(
                            "(s p) c -> p s c", p=128))
                    xn8 = p1xn.tile([128, GS, C], BF16, tag="xn8")
                    for j in range(GS):
                        jj = g * GS + j
                        if j < P1_DVE_J:
                            nc.vector.tensor_scalar(
                                xn8[:, j, :], xt8[:, j, :],
                                rr1f[:, jj:jj + 1], nm1f[:, jj:jj + 1],
                                ALU.mult, ALU.add)
                        else:
                            nc.scalar.activation(
                                xn8[:, j, :], xt8[:, j, :], AF.Identity,
                                bias=nm1f[:, jj:jj + 1],
                                scale=rr1f[:, jj:jj + 1])
                    nc.sync.dma_start_transpose(
                        xT[:, g * GS * 128:(g + 1) * GS * 128].rearrange(
                            "c (s p) -> c s p", s=GS), xn8[:])
                    for h2 in range(2):
                        p1b_unit(0, g, h2, p1ps, p1sig, p1pr, xT)
                _collective([ab_i[0][:]], [ab_o[0][:]])
                for g in range(NG):
                    for h2 in range(2):
                        p1b_unit(1, g, h2, p1ps, p1sig, p1pr, xT)
                nc.sync.dma_start(xT_d[:, :], xT[:])
                if dbg:
                    nc.sync.dma_start(dbg_xT[:, :], xT[:])

            # ================= Phase 2 =================
            KC = N // 128

            def p2_channel(q, s, p2a, p2b, p2o, p2mm, p2s, p2pt, identb):
                AT = p2a.tile([128, KC, N], BF16, tag="AT")
                BT = p2b.tile([128, KC, N], BF16, tag="BT")
                for d in range(NCORES):
                    for which, Tt in ((0, AT), (1, BT)):
                        stg = p2s.tile([R, N], BF16, tag="stg")
                        eng = nc.sync if (d + which) % 2 == 0 else nc.scalar
                        eng.dma_start(
                            stg[:],
                            ab_o[q][16 * d + 2 * s + which, :].rearrange(
                                "(i k) -> i k", i=R))
                        psT2 = p2pt.tile([128, KC, R], BF16, tag="psT2")
                        for kc in range(KC):
                            nc.tensor.transpose(
                                psT2[:, kc, :],
                                stg[:, kc * 128:(kc + 1) * 128],
                                identb[0:R, 0:R])
                        if (d + which) % 2 == 0:
                            nc.vector.tensor_copy(
                                Tt[:, :, d * R:(d + 1) * R], psT2[:])
                        else:
                            nc.scalar.activation(
                                Tt[:, :, d * R:(d + 1) * R], psT2[:],
                                AF.Copy)
                for ib in range(6):
                    ps2 = p2mm.tile([128, 2, 512], F32, tag="ps2")
                    for kc in range(KC):
                        lhsT = AT[:, kc, ib * 128:(ib + 1) * 128]
                        for jh in range(2):
                            nc.tensor.matmul(
                                ps2[:, jh, 0:384], lhsT,
                                BT[:, kc, jh * 384:(jh + 1) * 384],
                                start=(kc == 0), stop=(kc == KC - 1))
                    ot = p2o.tile([128, 2, 384], BF16, tag="ot")
                    if ib % 2 == 0:
                        nc.scalar.activation(ot[:], ps2[:, :, 0:384], AF.Copy)
                    else:
                        nc.vector.tensor_copy(ot[:], ps2[:, :, 0:384])
                    otv = ot[:].rearrange("i h j -> i (h j)")
                    i0 = ib * 128
                    while i0 < (ib + 1) * 128:
                        d, off = divmod(i0, R)
                        n = min(R - off, (ib + 1) * 128 - i0)
                        nc.scalar.dma_start(
                            o2_i[q][8 * d + s, :].rearrange(
                                "(i j) -> i j", i=R)[off:off + n, :],
                            otv[i0 - ib * 128:i0 - ib * 128 + n, :])
                        i0 += n

            with tc.tile_pool(name="p2a", bufs=3) as p2a, \
                 tc.tile_pool(name="p2b", bufs=3) as p2b, \
                 tc.tile_pool(name="p2o", bufs=4) as p2o, \
                 tc.tile_pool(name="p2s", bufs=6) as p2s, \
                 tc.tile_pool(name="p2c", bufs=1) as p2c, \
                 tc.tile_pool(name="p2pt", bufs=3, space="PSUM") as p2pt, \
                 tc.tile_pool(name="p2mm", bufs=2, space="PSUM") as p2mm:
                identb = p2c.tile([128, 128], BF16)
                from concourse.masks import make_identity
                make_identity(nc, identb[:])
                for s in range(8):
                    p2_channel(0, s, p2a, p2b, p2o, p2mm, p2s, p2pt, identb)
                _collective([ab_i[1][:]], [ab_o[1][:]])
                for s in range(8):
                    p2_channel(1, s, p2a, p2b, p2o, p2mm, p2s, p2pt, identb)
                _collective([o2_i[0][:]], [o2_o[0][:]])

            # ================= Phase 3 =================
            with tc.tile_pool(name="p3rr", bufs=1) as p3rr, \
                 tc.tile_pool(name="p3ps", bufs=2, space="PSUM") as p3ps:
                rr3 = p3rr.tile([128, NG, GS], F32)
                nm3 = p3rr.tile([128, NG, GS], F32)
                with tc.tile_pool(name="p3a", bufs=3) as p3a, \
                     tc.tile_pool(name="p3st", bufs=1) as p3st:
                    st3 = p3st.tile([128, 2, NG, GS, 6], F32)
                    for g in range(NG):
                        o2p = p3a.tile([128, GS, 64], BF16, tag="o2pa")
                        nc.sync.dma_start_transpose(
                            o2p[:],
                            o2_o[0][:, g * GS * 128:(g + 1) * GS * 128])
                        for j in range(GS):
                            nc.vector.bn_stats(st3[:, 0, g, j, :],
                                               o2p[:, j, :])
                    _collective([o2_i[1][:]], [o2_o[1][:]])
                    for g in range(NG):
                        o2p = p3a.tile([128, GS, 64], BF16, tag="o2pb")
                        nc.sync.dma_start_transpose(
                            o2p[:],
                            o2_o[1][:, g * GS * 128:(g + 1) * GS * 128])
                        for j in range(GS):
                            nc.vector.bn_stats(st3[:, 1, g, j, :],
                                               o2p[:, j, :])
                    sh = [128, NG, GS, 1]
                    sta = st3[:, 0]
                    stb = st3[:, 1]
                    stats_combine(
                        [sta[:, :, :, 1:2], sta[:, :, :, 4:5],
                         stb[:, :, :, 1:2], stb[:, :, :, 4:5]],
                        [sta[:, :, :, 2:3], sta[:, :, :, 5:6],
                         stb[:, :, :, 2:3], stb[:, :, :, 5:6]],
                        rr3[:].rearrange("p g (s o) -> p g s o", o=1),
                        nm3[:].rearrange("p g (s o) -> p g s o", o=1),
                        p3st, nc, epsb, sh)
                rr3f = rr3[:].rearrange("p g s -> p (g s)")
                nm3f = nm3[:].rearrange("p g s -> p (g s)")
                _cms = [tc.tile_pool(name="xTp3", bufs=1),
                        tc.tile_pool(name="p3b", bufs=2),
                        tc.tile_pool(name="p3n", bufs=2),
                        tc.tile_pool(name="p3sig", bufs=2),
                        tc.tile_pool(name="p3om", bufs=2)]
                xTp3, p3b, p3n, p3sig, p3om = (cm.__enter__()
                                               for cm in _cms)
                xT3 = xTp3.tile([C, POS], BF16)
                nc.sync.dma_start(xT3[:], xT_d[:, :])
                for g in range(NG):
                    o2p = p3b.tile([128, GS, 128], BF16, tag="o2p")
                    for q in range(2):
                        nc.sync.dma_start_transpose(
                            o2p[:, :, 64 * q:64 * q + 64],
                            o2_o[q][:, g * GS * 128:(g + 1) * GS * 128])
                    o2n = p3n.tile([128, GS, 128], BF16, tag="o2n")
                    for j in range(GS):
                        jj = g * GS + j
                        if j < P3_DVE_J:
                            nc.vector.tensor_scalar(
                                o2n[:, j, :], o2p[:, j, :],
                                rr3f[:, jj:jj + 1], nm3f[:, jj:jj + 1],
                                ALU.mult, ALU.add)
                        else:
                            nc.scalar.activation(
                                o2n[:, j, :], o2p[:, j, :], AF.Identity,
                                bias=nm3f[:, jj:jj + 1],
                                scale=rr3f[:, jj:jj + 1])
                    o2nT = p3n.tile([128, GS, 128], BF16, tag="o2nT")
                    nc.sync.dma_start_transpose(o2nT[:], o2n[:])
                    o2v = o2nT[:].rearrange("c s p -> c (s p)")
                    for h2 in range(2):
                        pos0 = g * 2048 + h2 * 1024
                        ps3 = p3ps.tile([128, 2, 2, 512], F32, tag="ps3")
                        for hb in range(2):
                            sl = slice(h2 * 1024 + hb * 512,
                                       h2 * 1024 + (hb + 1) * 512)
                            nc.tensor.matmul(ps3[:, 0, hb, :],
                                             wfsb[:, 0:C], o2v[:, sl],
                                             start=True, stop=True)
                            nc.tensor.matmul(ps3[:, 1, hb, :],
                                             wfsb[:, C:2 * C],
                                             xT3[:, pos0 + hb * 512:
                                                 pos0 + (hb + 1) * 512],
                                             start=True, stop=True)
                        sigb = p3sig.tile([128, 2, 512], BF16, tag="sigb")
                        nc.scalar.activation(sigb[:], ps3[:, 1, :, :],
                                             AF.Sigmoid, bias=cfsb[:, 1:2])
                        om = p3om.tile([128, 1024], F32, tag="om")
                        nc.vector.scalar_tensor_tensor(
                            om[:].rearrange("p (a b) -> p a b", a=2),
                            ps3[:, 0, :, :], cfsb[:, 0:1], sigb[:],
                            ALU.add, ALU.mult)
                        nc.scalar.dma_start(out_r[:, pos0:pos0 + 1024], om[:])
                for _cm in reversed(_cms):
                    _cm.__exit__(None, None, None)
            if dbg:
                nc.sync.dma_start(dbg_ab0i[:, :], ab_i[0][:, :])
                nc.sync.dma_start(dbg_o2i[:, :], o2_i[0][:, :])
    nc.compile()
    return nc


def _perm_pi():
    """p3 o2nT partition p -> einsum channel c."""
    p = np.arange(128)
    q = p // 64
    d = (p % 64) // 8
    s = p % 8
    return d * 16 + q * 8 + s


def _prep_weights(ln1_w, proj_w, gate_w, ln2_w, out_w, gating_w, ln1_b, ln2_b):
    p = np.arange(128)
    d, u = p // 16, p % 16
    w1 = np.zeros((C, 512), np.float32)
    c1 = np.zeros(512, np.float32)
    for q in range(2):
        c = d * 16 + q * 8 + u // 2
        row = 2 * c + (u % 2)
        w1[:, q * 256 + 0:q * 256 + 128] = (proj_w[row] * ln1_w).T
        w1[:, q * 256 + 128:q * 256 + 256] = (gate_w[row] * ln1_w).T
        c1[q * 256:q * 256 + 128] = proj_w[row] @ ln1_b
        c1[q * 256 + 128:q * 256 + 256] = gate_w[row] @ ln1_b
    pi = _perm_pi()
    wfo = (out_w * ln2_w[None, :]).T[pi]       # rows permuted to o2nT order
    wfg = (gating_w * ln1_w[None, :]).T
    wf = np.concatenate([wfo, wfg], axis=1)
    cf = np.concatenate([out_w @ ln2_b, gating_w @ ln1_b])
    return w1, c1.reshape(1, 512), wf, cf.reshape(1, 256)


def kernel(pair, mask, ln1_w, ln1_b, proj_w, gate_w, ln2_w, ln2_b, out_w,
           gating_w):
    pair = np.asarray(pair, dtype=np.float32)
    mask = np.asarray(mask, dtype=np.float32)
    ln1_w = np.asarray(ln1_w, np.float32); ln1_b = np.asarray(ln1_b, np.float32)
    ln2_w = np.asarray(ln2_w, np.float32); ln2_b = np.asarray(ln2_b, np.float32)
    proj_w = np.asarray(proj_w, np.float32)
    gate_w = np.asarray(gate_w, np.float32)
    out_w = np.asarray(out_w, np.float32)
    gating_w = np.asarray(gating_w, np.float32)

    use_mask = not bool(np.all(mask == 1.0))
    if use_mask not in _PROGRAM_CACHE:
        _PROGRAM_CACHE[use_mask] = _build_program(use_mask)
    nc = _PROGRAM_CACHE[use_mask]

    w1, c1t, wf, cft = _prep_weights(ln1_w, proj_w, gate_w, ln2_w, out_w,
                                     gating_w, ln1_b, ln2_b)
    bf = ml_dtypes.bfloat16
    pair_b = pair.astype(bf).reshape(NCORES, POS, C)
    w1_b = np.ascontiguousarray(w1).astype(bf)
    wf_b = np.ascontiguousarray(wf).astype(bf)

    in_maps = []
    for c in range(NCORES):
        m = {
            "pair_r": pair_b[c],
            "w1t": w1_b,
            "wfin": wf_b,
            "c1r": c1t,
            "cfr": cft,
        }
        if use_mask:
            m["mask_r"] = np.ascontiguousarray(
                mask[c * R:(c + 1) * R].reshape(POS // 1024, 1024))
        in_maps.append(m)

    trace = os.environ.get("TRIMUL_TRACE", "") == "1"
    res = run_bass_kernel_spmd(nc, in_maps, core_ids=list(range(NCORES)),
                               trace=trace)
    global LAST_EXEC_NS, LAST_TRACE
    if res.exec_time_ns is not None:
        LAST_EXEC_NS = res.exec_time_ns
    if res.instructions_and_trace is not None:
        LAST_TRACE = res.instructions_and_trace[1]
    out = np.empty((N, N, C), np.float32)
    for c in range(NCORES):
        out[c * R:(c + 1) * R] = res.results[c]["out_r"].T.reshape(R, N, C)
    return out
